# revision 1
# baseline (speedup 1.0000x reference)
"""GAT layer kernel for Trainium2, 8 NeuronCores.

Strategy (src-range sharding, no collectives):
  - Host: sort edges by src node; core k owns src nodes
    [k*nodes_per_core, (k+1)*nodes_per_core).  Within a core, nodes are
    tiled 128 at a time; each tile's edges are padded to C chunks of 128
    edge slots (C = global max, data-derived).
  - Device phase 1: build Whaug table [Npad, 260] fp16 via one fused matmul
    x @ [W_h0 | W_h1 | w_s0 w_s1 w_t0 w_t1] (fp16 inputs, fp32 PSUM).
    Table row: [Wh_h0(128), 1.0, Wh_h1(128), 1.0, t0, t1, pad(2)].
    Also writes st_tab [Npad, 2] fp32 = per-node (s0, s1).
  - Device phase 2: per 128-edge chunk, [128,1]-offset indirect-DMA
    gathers of the dst rows (520B, t rides along) and the s[src] pairs;
    e = leaky_relu(s+t); p = exp(e - SHIFT).  Then a one-hot matmul
    (onehot[e, src_local]^T @ (p * G)) accumulates both the aggregation
    numerator and the softmax denominator (via the 1.0 columns in the
    table) into PSUM [128 src, 258].  Finalize: out = num/den, or the
    node's own Wh row where den==0 (deg-0 nodes).
"""

import math
import sys
from dataclasses import dataclass

import numpy as np

sys.path.insert(0, "/opt/trn_rl_repo")

import concourse.bass as bass
import concourse.mybir as mybir
import concourse.tile as tile
from concourse import bacc
from concourse.bass import IndirectOffsetOnAxis
from concourse.masks import make_identity
from concourse.bass_utils import run_bass_kernel_spmd

# Problem shapes (fixed by the graded problem)
N_NODES = 50000
IN_DIM = 256
OUT_DIM = 128
NUM_HEADS = 2

P = 128
AUGW = 264  # table row: Wh0(128),1,Wh1(128),1,t0,t1,s0,s1,pad(2)
RHSW = 258  # columns fed to the aggregation matmul
SHIFT = 4.0  # constant subtracted inside exp (softmax-invariant)

F32 = mybir.dt.float32
F32R = mybir.dt.float32r
F16 = mybir.dt.float16
I32 = mybir.dt.int32


@dataclass(frozen=True)
class Cfg:
    n_nodes: int
    n_cores: int
    C: int
    span_tiles: int = 16
    build_f16: bool = True
    reps: int = 1

    @property
    def nodes_per_core(self):
        return self.n_nodes // self.n_cores

    @property
    def ntiles(self):
        return (self.nodes_per_core + P - 1) // P

    @property
    def npad(self):
        return self.n_cores * self.ntiles * P


def _ap_expand(ap, dims):
    """Return an AP keeping ap's partition dim and replacing the free dims
    with `dims` = list of (step, count) pairs (element units)."""
    return bass.AP(ap.tensor, ap.offset, [list(ap.ap[0])] + [[s, c] for s, c in dims])


def host_prep(x, edge_index, W_w, W_b, a, n_cores=8):
    """Pure index/layout preprocessing + parameter folding. Returns
    (cfg, shared_inputs, per_core_inputs)."""
    x = np.asarray(x, dtype=np.float32)
    edge_index = np.asarray(edge_index)
    W_w = np.asarray(W_w, dtype=np.float32)
    W_b = np.asarray(W_b, dtype=np.float32)
    a = np.asarray(a, dtype=np.float32)
    assert np.abs(W_b).max() == 0.0, "nonzero bias not supported"

    n_nodes, in_dim = x.shape
    D = OUT_DIM
    n_edges = edge_index.shape[1]

    # Parameter folding: per-head score vectors (weight preprocessing).
    a_src, a_dst = a[:D], a[D:]
    ws0 = W_w[:, 0:D] @ a_src
    ws1 = W_w[:, D : 2 * D] @ a_src
    wt0 = W_w[:, 0:D] @ a_dst
    wt1 = W_w[:, D : 2 * D] @ a_dst
    wbig = np.concatenate(
        [W_w, ws0[:, None], ws1[:, None], wt0[:, None], wt1[:, None]], axis=1
    ).astype(np.float32)  # [in_dim, 260]

    # ---- edge index preprocessing ----
    src = np.asarray(edge_index[0], dtype=np.int64)
    dst = np.asarray(edge_index[1], dtype=np.int64)
    order = np.argsort(src, kind="stable")
    src_s = src[order]
    dst_s = dst[order].astype(np.int32)

    npc = n_nodes // n_cores
    ntiles = (npc + P - 1) // P

    # Degree-balanced global node->(core,tile,slot) assignment (LPT over
    # all tiles): equalizes per-tile edge counts so C shrinks and cores
    # stay balanced.
    import heapq

    ntile_tot = n_cores * ntiles
    deg_all = np.bincount(src, minlength=n_nodes)
    order_n = np.argsort(-deg_all, kind="stable")
    heap = [(0, t) for t in range(ntile_tot)]
    heapq.heapify(heap)
    fill = np.zeros(ntile_tot, dtype=np.int64)
    node_tile = np.zeros(n_nodes, dtype=np.int64)
    node_slot = np.zeros(n_nodes, dtype=np.int64)
    for n in order_n:
        while True:
            w, t = heapq.heappop(heap)
            if fill[t] < P:
                break
        node_tile[n] = t
        node_slot[n] = fill[t]
        fill[t] += 1
        if fill[t] < P:
            heapq.heappush(heap, (w + int(deg_all[n]), t))

    gtile = node_tile[src_s]
    pos = node_slot[src_s].astype(np.float32)
    # regroup edges by tile
    order2 = np.argsort(gtile, kind="stable")
    gtile = gtile[order2]
    dst_s = dst_s[order2]
    src_s = src_s[order2]
    pos = pos[order2]

    counts = np.bincount(gtile, minlength=ntile_tot)
    C = int(math.ceil(counts.max() / P))
    cfg = Cfg(n_nodes=n_nodes, n_cores=n_cores, C=C)
    npad = cfg.npad
    slots_per_tile = C * P

    bdt = np.float16 if cfg.build_f16 else np.float32
    xT = np.zeros((in_dim, npad), dtype=bdt)
    xT[:, :n_nodes] = x.T.astype(bdt)
    wbig = wbig.astype(bdt)
    iota = np.broadcast_to(np.arange(P, dtype=np.float32), (P, P)).copy()

    starts = np.zeros(ntile_tot, dtype=np.int64)
    starts[1:] = np.cumsum(counts)[:-1]
    slot_in_tile = np.arange(n_edges) - starts[gtile]

    # Slot s of tile t maps to (chunk j = s // P, partition p = s % P).
    dstI = np.zeros((ntile_tot, slots_per_tile), dtype=np.int32)
    srcI = np.full((ntile_tot, slots_per_tile), npad - 1, dtype=np.int32)
    srcL = np.full((ntile_tot, slots_per_tile), -1.0, dtype=np.float32)
    flat = gtile * slots_per_tile + slot_in_tile
    dstI.reshape(-1)[flat] = dst_s
    srcI.reshape(-1)[flat] = src_s.astype(np.int32)
    srcL.reshape(-1)[flat] = pos

    # [tiles, C, P] -> per-core [P, ntiles*C] (partition-major SBUF layout)
    def to_core_layout(arr):
        a4 = arr.reshape(n_cores, ntiles, C, P)
        return np.ascontiguousarray(np.transpose(a4, (0, 3, 1, 2))).reshape(
            n_cores, P, ntiles * C
        )

    dstI_c = to_core_layout(dstI)
    srcI_c = to_core_layout(srcI)
    srcL_c = to_core_layout(srcL)

    # own global node id per (p, t): the node sitting at tile t, slot p
    # (pad slots -> node 0, harmless)
    own_global = np.zeros((ntile_tot, P), dtype=np.int32)
    own_global[node_tile, node_slot] = np.arange(n_nodes, dtype=np.int32)

    shared = {"xT": xT, "wbig": wbig, "iota": iota}
    per_core = []
    for k in range(n_cores):
        tl = slice(k * ntiles, (k + 1) * ntiles)
        ownI = np.ascontiguousarray(own_global[tl].T).astype(np.int32)
        mine = (node_tile >= k * ntiles) & (node_tile < (k + 1) * ntiles)
        nodes_k = np.nonzero(mine)[0]
        rows_k = (node_tile[nodes_k] - k * ntiles) * P + node_slot[nodes_k]
        per_core.append(
            {"dstI": dstI_c[k], "srcI": srcI_c[k], "srcL": srcL_c[k], "ownI": ownI,
             "_nodes": nodes_k, "_rows": rows_k}
        )
    return cfg, shared, per_core


def build_program(cfg: Cfg):
    """Build the Bass/Tile program (identical across cores)."""
    C, ntiles, npad = cfg.C, cfg.ntiles, cfg.npad
    BDT = F16 if cfg.build_f16 else F32
    nc = bacc.Bacc("TRN2", target_bir_lowering=False, debug=False)

    xT_d = nc.dram_tensor("xT", [IN_DIM, npad], BDT, kind="ExternalInput")
    wbig_d = nc.dram_tensor("wbig", [IN_DIM, 260], BDT, kind="ExternalInput")
    iota_d = nc.dram_tensor("iota", [P, P], F32, kind="ExternalInput")
    dstI_d = nc.dram_tensor("dstI", [P, ntiles * C], I32, kind="ExternalInput")
    srcI_d = nc.dram_tensor("srcI", [P, ntiles * C], I32, kind="ExternalInput")
    srcL_d = nc.dram_tensor("srcL", [P, ntiles * C], F32, kind="ExternalInput")
    ownI_d = nc.dram_tensor("ownI", [P, ntiles], I32, kind="ExternalInput")
    out_d = nc.dram_tensor("out", [ntiles * P, 2 * OUT_DIM], F32, kind="ExternalOutput")

    whaug_d = nc.dram_tensor("whaug", [npad, AUGW], F16)

    n_alltiles = npad // P

    with tile.TileContext(nc) as tc:
        with (
            tc.tile_pool(name="const", bufs=1) as constp,
            tc.tile_pool(name="xk", bufs=2) as xkp,
            tc.tile_pool(name="bld_ps", bufs=2, space="PSUM") as bldps,
            tc.tile_pool(name="augg", bufs=2) as auggp,
            tc.tile_pool(name="gall", bufs=3) as gallp,
            tc.tile_pool(name="oneh", bufs=2) as onehp,
            tc.tile_pool(name="rhs", bufs=2) as rhsp,
            tc.tile_pool(name="agg_ps", bufs=3, space="PSUM") as aggps,
            tc.tile_pool(name="tr_ps", bufs=2, space="PSUM") as trps,
            tc.tile_pool(name="s_ps", bufs=1, space="PSUM") as spsp,
            tc.tile_pool(name="fin", bufs=3) as finp,
            tc.tile_pool(name="og", bufs=2) as ogp,
        ):
            # ---------------- constants / index loads ----------------
            wb = constp.tile([P, 2, 260], BDT, tag="wb")
            nc.sync.dma_start(
                out=wb[:],
                in_=wbig_d[:, :].rearrange("(kt kp) c -> kp kt c", kp=P),
            )
            iota_t = constp.tile([P, P], F32, tag="iota")
            nc.sync.dma_start(out=iota_t[:], in_=iota_d[:, :])
            dstI_t = constp.tile([P, ntiles * C], I32, tag="dstI")
            nc.sync.dma_start(out=dstI_t[:], in_=dstI_d[:, :])
            srcI_t = constp.tile([P, ntiles * C], I32, tag="srcI")
            nc.sync.dma_start(out=srcI_t[:], in_=srcI_d[:, :])
            srcL_t = constp.tile([P, ntiles * C], F32, tag="srcL")
            nc.sync.dma_start(out=srcL_t[:], in_=srcL_d[:, :])
            ownI_t = constp.tile([P, ntiles], I32, tag="ownI")
            nc.sync.dma_start(out=ownI_t[:], in_=ownI_d[:, :])
            shift_t = constp.tile([P, 1], F32, tag="shift")
            nc.vector.memset(shift_t[:], -SHIFT)
            ident_t = constp.tile([P, P], F16, tag="ident")
            make_identity(nc, ident_t[:])

            # ---------------- phase 1: build whaug + sttab ----------------
            GRP = 8  # node tiles per table write group
            n0 = 0
            while n0 < n_alltiles:
                span = min(cfg.span_tiles, n_alltiles - n0)
                xk = xkp.tile([P, 2, cfg.span_tiles * P], BDT, tag="xk")
                for kt in range(2):
                    nc.sync.dma_start(
                        out=xk[:, kt, 0 : span * P],
                        in_=xT_d[kt * P : (kt + 1) * P, n0 * P : (n0 + span) * P],
                    )
                g0 = 0
                while g0 < span:
                    grp = min(GRP, span - g0)
                    aug = auggp.tile([P, GRP, AUGW], F16, tag="aug")
                    nc.vector.memset(aug[:], 1.0)
                    for g in range(grp):
                        nt = g0 + g
                        ps = bldps.tile([P, 260], F32, tag="bld")
                        for kt in range(2):
                            nc.tensor.matmul(
                                out=ps[:],
                                lhsT=xk[:, kt, nt * P : (nt + 1) * P],
                                rhs=wb[:, kt, :],
                                start=(kt == 0),
                                stop=(kt == 1),
                            )
                        nc.vector.tensor_copy(
                            out=aug[:, g, 0:OUT_DIM], in_=ps[:, 0:OUT_DIM]
                        )
                        nc.vector.tensor_copy(
                            out=aug[:, g, OUT_DIM + 1 : 2 * OUT_DIM + 1],
                            in_=ps[:, OUT_DIM : 2 * OUT_DIM],
                        )
                        nc.vector.tensor_copy(
                            out=aug[:, g, 2 * OUT_DIM + 2 : 2 * OUT_DIM + 4],
                            in_=ps[:, 2 * OUT_DIM + 2 : 2 * OUT_DIM + 4],
                        )
                        nc.vector.tensor_copy(
                            out=aug[:, g, 2 * OUT_DIM + 4 : 2 * OUT_DIM + 6],
                            in_=ps[:, 2 * OUT_DIM : 2 * OUT_DIM + 2],
                        )
                    r0 = (n0 + g0) * P
                    nc.sync.dma_start(
                        out=whaug_d[r0 : r0 + grp * P, :].rearrange(
                            "(g p) c -> p g c", p=P
                        ),
                        in_=aug[:, 0:grp, :],
                    )
                    g0 += grp
                n0 += span

            # ---------------- phase 2: attention + aggregation ----------------
            OGRP = 8
            og = None
            for t in [tt for _ in range(cfg.reps) for tt in range(ntiles)]:
                g = t % OGRP
                if g == 0:
                    og = ogp.tile([P, OGRP, 2 * OUT_DIM], F32, tag="og")

                fb = finp.tile([P, AUGW], F16, tag="fb")
                nc.gpsimd.indirect_dma_start(
                    out=fb[:],
                    out_offset=None,
                    in_=whaug_d[:, :],
                    in_offset=IndirectOffsetOnAxis(ap=ownI_t[:, t : t + 1], axis=0),
                )
                gall = gallp.tile([P, C, AUGW], F16, tag="gall")
                for c in range(C):
                    nc.gpsimd.indirect_dma_start(
                        out=gall[:, c, :],
                        out_offset=None,
                        in_=whaug_d[:, :],
                        in_offset=IndirectOffsetOnAxis(
                            ap=dstI_t[:, t * C + c : t * C + c + 1], axis=0
                        ),
                    )
                # onehot built early: also used (transposed on PE) to expand s
                oneh = onehp.tile([P, C, P], F16, tag="oneh")
                srcL_sl = srcL_t[:, t * C : (t + 1) * C]
                nc.vector.tensor_tensor(
                    out=oneh[:],
                    in0=_ap_expand(srcL_sl, [(1, C), (0, P)]),
                    in1=_ap_expand(iota_t[:], [(0, C), (1, P)]),
                    op=mybir.AluOpType.is_equal,
                )
                s_ps = spsp.tile([P, C, 2], F32, tag="s_ps")
                for c in range(C):
                    oneT_ps = trps.tile([P, P], F16, tag="oneT_ps")
                    nc.tensor.transpose(
                        out=oneT_ps[:], in_=oneh[:, c, :], identity=ident_t[:]
                    )
                    oneT = finp.tile([P, P], F16, tag="oneT")
                    nc.vector.tensor_copy(out=oneT[:], in_=oneT_ps[:])
                    nc.tensor.matmul(
                        out=s_ps[:, c, :],
                        lhsT=oneT[:],
                        rhs=fb[:, 2 * OUT_DIM + 4 : 2 * OUT_DIM + 6],
                        start=True,
                        stop=True,
                    )
                # e = s + t ; lrelu ; p = exp(e - SHIFT)  (per tile batch)
                e_t = finp.tile([P, C * 2], F32, tag="e_t")
                nc.vector.tensor_tensor(
                    out=e_t[:],
                    in0=s_ps[:],
                    in1=gall[:, :, RHSW : RHSW + 2],
                    op=mybir.AluOpType.add,
                )
                e_s = finp.tile([P, C * 2], F32, tag="e_s")
                nc.vector.tensor_scalar(
                    out=e_s[:], in0=e_t[:], scalar1=0.2, scalar2=None,
                    op0=mybir.AluOpType.mult,
                )
                lr_t = finp.tile([P, C * 2], F32, tag="lr_t")
                nc.vector.tensor_tensor(
                    out=lr_t[:], in0=e_t[:], in1=e_s[:], op=mybir.AluOpType.max,
                )
                p16 = finp.tile([P, C, 2], F16, tag="p16")
                nc.scalar.activation(
                    out=p16[:].rearrange("p c h -> p (c h)"),
                    in_=lr_t[:],
                    func=mybir.ActivationFunctionType.Exp,
                    bias=shift_t[:, 0:1],
                )
                rhs = rhsp.tile([P, C, RHSW], F16, tag="rhs")
                nc.vector.tensor_tensor(
                    out=rhs[:],
                    in0=gall[:, :, 0:RHSW],
                    in1=_ap_expand(p16[:], [(2, C), (1, 2), (0, OUT_DIM + 1)]),
                    op=mybir.AluOpType.mult,
                )
                ps = aggps.tile([P, RHSW], F32, tag="agg")
                for c in range(C):
                    nc.tensor.matmul(
                        out=ps[:],
                        lhsT=oneh[:, c, :],
                        rhs=rhs[:, c, :],
                        start=(c == 0),
                        stop=(c == C - 1),
                    )

                # ---- finalize tile t ----
                den_ap = _ap_expand(ps[:], [(OUT_DIM + 1, 2)])
                den_ap = bass.AP(den_ap.tensor, den_ap.offset + OUT_DIM, den_ap.ap)
                dns = finp.tile([P, 2], F32, tag="dns")
                nc.vector.tensor_scalar(
                    out=dns[:], in0=den_ap, scalar1=1e-30, scalar2=None,
                    op0=mybir.AluOpType.max,
                )
                rcp = finp.tile([P, 2], F32, tag="rcp")
                nc.vector.reciprocal(out=rcp[:], in_=dns[:])
                nmask = finp.tile([P, 1], F32, tag="nmask")
                nc.vector.tensor_scalar(
                    out=nmask[:], in0=ps[:, OUT_DIM : OUT_DIM + 1], scalar1=0.0,
                    scalar2=None, op0=mybir.AluOpType.is_le,
                )
                for h in range(2):
                    nc.vector.tensor_scalar(
                        out=og[:, g, h * OUT_DIM : (h + 1) * OUT_DIM],
                        in0=ps[:, h * (OUT_DIM + 1) : h * (OUT_DIM + 1) + OUT_DIM],
                        scalar1=rcp[:, h : h + 1],
                        scalar2=None,
                        op0=mybir.AluOpType.mult,
                    )
                # fallback rows (deg == 0): out += nmask * Wh(own row)
                fbm = finp.tile([P, 2, OUT_DIM], F32, tag="fbm")
                nc.vector.tensor_scalar(
                    out=fbm[:],
                    in0=_ap_expand(fb[:], [(OUT_DIM + 1, 2), (1, OUT_DIM)]),
                    scalar1=nmask[:, 0:1],
                    scalar2=None,
                    op0=mybir.AluOpType.mult,
                )
                nc.vector.tensor_tensor(
                    out=og[:, g, :],
                    in0=og[:, g, :],
                    in1=fbm[:].rearrange("p a b -> p (a b)"),
                    op=mybir.AluOpType.add,
                )

                if g == OGRP - 1 or t == ntiles - 1:
                    t0 = t - g
                    nc.sync.dma_start(
                        out=out_d[t0 * P : (t + 1) * P, :].rearrange(
                            "(g p) c -> p g c", p=P
                        ),
                        in_=og[:, 0 : g + 1, :],
                    )

    nc.compile()
    return nc


_prog_cache = {}


def kernel(x, edge_index, W_w, W_b, a):
    cfg, shared, per_core = host_prep(x, edge_index, W_w, W_b, a, n_cores=8)
    if cfg not in _prog_cache:
        _prog_cache[cfg] = build_program(cfg)
    nc = _prog_cache[cfg]
    in_maps = [
        {kk: v for kk, v in {**shared, **pc}.items() if not kk.startswith("_")}
        for pc in per_core
    ]
    res = run_bass_kernel_spmd(nc, in_maps, list(range(cfg.n_cores)))
    out = np.zeros((cfg.n_nodes, 2 * OUT_DIM), dtype=np.float32)
    for k in range(cfg.n_cores):
        pc = per_core[k]
        out[pc["_nodes"]] = res.results[k]["out"][pc["_rows"]]
    return out



# revision 10
# speedup vs baseline: 1.1037x; 1.1037x over previous
"""GAT layer kernel for Trainium2, 8 NeuronCores.

Strategy (src-range sharding, no collectives):
  - Host: LPT-balance src nodes over 392 global (core, tile) slots so each
    tile owns ~2048 edges (C=16 chunks of 128).  Table rows live in
    per-core ROTATED tile-slot space: core k's own nodes occupy rows
    [0, ntiles*128) so phase-2 fallback/s data comes straight from SBUF.
  - Phase 1 (device): whaug table [npad, 264] fp16 rows
    [Wh0(128) | Wh1(128) | t0 t1 s0 s1 | pad2] via x @ [W | wt | ws]
    matmuls; PSUM->fp16 conversion on the Activation engine (one copy per
    tile); own-tile rows also kept in SBUF (fb_all) + s-pairs (s_all).
  - Phase 2 (device), per tile:
      Pool : C indirect row-gathers (dst rows, 128 offsets each)
      PE   : srcL broadcast (ones @ srcL_row), C tiny s-expand matmuls,
             C aggregation matmuls (one-hot lhsT)
      DVE  : one-hot builds (oneh [e,slot], oneT [slot,e]), rhs scaling by
             p, tiny e-ops; reciprocal
      ACT  : exp, output scaling
    Softmax uses a global SHIFT (softmax-invariant); deg-0 fallback uses
    out = (num + d*Wh_own) / (den + d) with d=1e-30 (exact for den=0).
"""

import math
import sys
from dataclasses import dataclass

import numpy as np

sys.path.insert(0, "/opt/trn_rl_repo")

import concourse.bass as bass
import concourse.mybir as mybir
import concourse.tile as tile
from concourse import bacc
from concourse.bass import IndirectOffsetOnAxis
from concourse.bass_utils import run_bass_kernel_spmd

N_NODES = 50000
IN_DIM = 256
OUT_DIM = 128
NUM_HEADS = 2

P = 128
AUGW = 260  # table row: Wh0(128), Wh1(128), t0, t1, s0, s1
USEW = 260
RHSW = 258  # aggregation matmul width: 256 features + 2 denominator cols
SHIFT = 4.0
DELTA = 1e-30

F32 = mybir.dt.float32
F16 = mybir.dt.float16
I32 = mybir.dt.int32


@dataclass(frozen=True)
class Cfg:
    n_nodes: int
    n_cores: int
    C: int
    span_tiles: int = 16
    reps: int = 1

    @property
    def nodes_per_core(self):
        return self.n_nodes // self.n_cores

    @property
    def ntiles(self):
        return (self.nodes_per_core + P - 1) // P

    @property
    def npad(self):
        return self.n_cores * self.ntiles * P


def _ap_expand(ap, dims):
    """Return an AP keeping ap's partition dim and replacing the free dims
    with `dims` = list of (step, count) pairs (element units)."""
    return bass.AP(ap.tensor, ap.offset, [list(ap.ap[0])] + [[s, c] for s, c in dims])


def host_prep(x, edge_index, W_w, W_b, a, n_cores=8):
    """Index/layout preprocessing + parameter folding."""
    x = np.asarray(x, dtype=np.float32)
    edge_index = np.asarray(edge_index)
    W_w = np.asarray(W_w, dtype=np.float32)
    W_b = np.asarray(W_b, dtype=np.float32)
    a = np.asarray(a, dtype=np.float32)
    assert np.abs(W_b).max() == 0.0, "nonzero bias not supported"

    n_nodes, in_dim = x.shape
    D = OUT_DIM
    n_edges = edge_index.shape[1]

    # wbig columns: [W (256) | wt0 wt1 | ws0 ws1]  ->  ps = [Wh0 Wh1 t0 t1 s0 s1]
    a_src, a_dst = a[:D], a[D:]
    ws0 = W_w[:, 0:D] @ a_src
    ws1 = W_w[:, D : 2 * D] @ a_src
    wt0 = W_w[:, 0:D] @ a_dst
    wt1 = W_w[:, D : 2 * D] @ a_dst
    wbig = np.concatenate(
        [W_w, wt0[:, None], wt1[:, None], ws0[:, None], ws1[:, None]], axis=1
    ).astype(np.float16)  # [in_dim, 260]

    src = np.asarray(edge_index[0], dtype=np.int64)
    dst = np.asarray(edge_index[1], dtype=np.int64)
    order = np.argsort(src, kind="stable")
    src_s = src[order]
    dst_s = dst[order].astype(np.int64)

    npc = n_nodes // n_cores
    ntiles = (npc + P - 1) // P

    # LPT: assign nodes to (core,tile,slot), balancing per-tile edge counts.
    import heapq

    ntile_tot = n_cores * ntiles
    deg_all = np.bincount(src, minlength=n_nodes)
    order_n = np.argsort(-deg_all, kind="stable")
    heap = [(0, t) for t in range(ntile_tot)]
    heapq.heapify(heap)
    fill = np.zeros(ntile_tot, dtype=np.int64)
    node_tile = np.zeros(n_nodes, dtype=np.int64)
    node_slot = np.zeros(n_nodes, dtype=np.int64)
    for n in order_n:
        while True:
            w, t = heapq.heappop(heap)
            if fill[t] < P:
                break
        node_tile[n] = t
        node_slot[n] = fill[t]
        fill[t] += 1
        if fill[t] < P:
            heapq.heappush(heap, (w + int(deg_all[n]), t))

    grow = node_tile * P + node_slot  # global tile-slot row of each node

    gtile = node_tile[src_s]
    pos = node_slot[src_s]
    order2 = np.argsort(gtile, kind="stable")
    gtile = gtile[order2]
    dst_s = dst_s[order2]
    pos = pos[order2]

    counts = np.bincount(gtile, minlength=ntile_tot)
    C = int(math.ceil(counts.max() / P))
    cfg = Cfg(n_nodes=n_nodes, n_cores=n_cores, C=C)
    npad = cfg.npad
    slots_per_tile = C * P

    # xT in global tile-slot space (empty slots -> 0 rows)
    xTslot = np.zeros((in_dim, npad), dtype=np.float16)
    xTslot[:, grow] = x.T.astype(np.float16)

    starts = np.zeros(ntile_tot, dtype=np.int64)
    starts[1:] = np.cumsum(counts)[:-1]
    slot_in_tile = np.arange(n_edges) - starts[gtile]

    # per-edge-slot arrays in (tile, chunk, partition) space
    dstG = np.zeros((ntile_tot, slots_per_tile), dtype=np.int64)  # dst grow
    srcL = np.full((ntile_tot, slots_per_tile), -1.0, dtype=np.float16)
    flat = gtile * slots_per_tile + slot_in_tile
    dstG.reshape(-1)[flat] = grow[dst_s]
    srcL.reshape(-1)[flat] = pos.astype(np.float16)

    def to_core_layout(arr, fillval):
        # [tiles, C*P] -> per-core [P, ntiles*C]
        a4 = arr.reshape(n_cores, ntiles, C, P)
        return np.ascontiguousarray(np.transpose(a4, (0, 3, 1, 2))).reshape(
            n_cores, P, ntiles * C
        )

    dstG_c = to_core_layout(dstG, 0)
    srcL_c = to_core_layout(srcL, -1.0)

    iota16 = np.broadcast_to(
        np.arange(P, dtype=np.float16), (P, P)
    ).copy()  # iota16[p, j] = j
    iota_col = np.arange(P, dtype=np.float16).reshape(P, 1).copy()

    shared = {"wbig": wbig, "iota16": iota16, "iotac": iota_col}
    per_core = []
    rows_pc = ntiles * P
    # srcL row layout for the PE broadcast: [1, ntiles*C*P]
    for k in range(n_cores):
        shift = k * rows_pc
        xT_k = np.roll(xTslot, -shift, axis=1)
        dstI_k = ((dstG_c[k] - shift) % npad).astype(np.int32)
        srcL_k = srcL_c[k]
        # row layout value at (t, c, p) = srcL_k[p, t*C + c]
        srcR_k = np.ascontiguousarray(
            np.transpose(srcL_k.reshape(P, ntiles, C), (1, 2, 0))
        ).reshape(1, ntiles * C * P)
        mine = (node_tile >= k * ntiles) & (node_tile < (k + 1) * ntiles)
        nodes_k = np.nonzero(mine)[0]
        rows_k = (node_tile[nodes_k] - k * ntiles) * P + node_slot[nodes_k]
        per_core.append(
            {
                "xT": xT_k,
                "dstI": dstI_k,
                "srcL": srcL_k,
                "srcR": srcR_k,
                "_nodes": nodes_k,
                "_rows": rows_k,
            }
        )
    return cfg, shared, per_core


def build_program(cfg: Cfg, marks=None):
    C, ntiles, npad = cfg.C, cfg.ntiles, cfg.npad

    nc = bacc.Bacc("TRN2", target_bir_lowering=False, debug=False)

    def _mark(label):
        if marks is not None:
            marks[label] = sum(len(b.instructions) for b in nc.m.functions[0].blocks)

    xT_d = nc.dram_tensor("xT", [IN_DIM, npad], F16, kind="ExternalInput")
    wbig_d = nc.dram_tensor("wbig", [IN_DIM, USEW], F16, kind="ExternalInput")
    iota16_d = nc.dram_tensor("iota16", [P, P], F16, kind="ExternalInput")
    iotac_d = nc.dram_tensor("iotac", [P, 1], F16, kind="ExternalInput")
    dstI_d = nc.dram_tensor("dstI", [P, ntiles * C], I32, kind="ExternalInput")
    srcL_d = nc.dram_tensor("srcL", [P, ntiles * C], F16, kind="ExternalInput")
    srcR_d = nc.dram_tensor("srcR", [1, ntiles * C * P], F16, kind="ExternalInput")
    out_d = nc.dram_tensor("out", [ntiles * P, 2 * OUT_DIM], F32, kind="ExternalOutput")

    whaug_d = nc.dram_tensor("whaug", [npad, AUGW], F16)

    n_alltiles = npad // P
    HB = C // 2  # chunks per srcl-broadcast half

    with tile.TileContext(nc) as tc:
        with (
            tc.tile_pool(name="const", bufs=1) as constp,
            tc.tile_pool(name="xk", bufs=2) as xkp,
            tc.tile_pool(name="bld_ps", bufs=2, space="PSUM") as bldps,
            tc.tile_pool(name="augg", bufs=2) as auggp,
            tc.tile_pool(name="own", bufs=1) as ownp,
            tc.tile_pool(name="gall", bufs=2) as gallp,
            tc.tile_pool(name="srcr", bufs=2) as srcrp,
            tc.tile_pool(name="oneh", bufs=2) as onehp,
            tc.tile_pool(name="rhs", bufs=2) as rhsp,
            tc.tile_pool(name="bc_ps", bufs=1, space="PSUM") as bcps,
            tc.tile_pool(name="s_ps", bufs=2, space="PSUM") as spsp,
            tc.tile_pool(name="agg_ps", bufs=2, space="PSUM") as aggps,
            tc.tile_pool(name="fin", bufs=2) as finp,
            tc.tile_pool(name="og", bufs=2) as ogp,
        ):
            # ---------------- constants ----------------
            wb = constp.tile([P, 2, USEW], F16, tag="wb")
            nc.sync.dma_start(
                out=wb[:], in_=wbig_d[:, :].rearrange("(kt kp) c -> kp kt c", kp=P)
            )
            iota16_t = constp.tile([P, P], F16, tag="iota16")
            nc.sync.dma_start(out=iota16_t[:], in_=iota16_d[:, :])
            iotac_t = constp.tile([P, 1], F16, tag="iotac")
            nc.sync.dma_start(out=iotac_t[:], in_=iotac_d[:, :])
            dstI_t = constp.tile([P, ntiles * C], I32, tag="dstI")
            nc.sync.dma_start(out=dstI_t[:], in_=dstI_d[:, :])
            srcL_t = constp.tile([P, ntiles * C], F16, tag="srcL")
            nc.sync.dma_start(out=srcL_t[:], in_=srcL_d[:, :])
            ones1_t = constp.tile([1, P], F16, tag="ones1")
            nc.vector.memset(ones1_t[:], 1.0)
            shift_t = constp.tile([P, 1], F32, tag="shift")
            nc.vector.memset(shift_t[:], -SHIFT)
            fb_all = ownp.tile([P, ntiles, 2 * OUT_DIM], F16, tag="fb_all")
            s_all = ownp.tile([P, ntiles, 2], F16, tag="s_all")

            _mark("consts_end")

            # ---------------- phase 1: build whaug table ----------------
            GRP = 8
            n0 = 0
            while n0 < n_alltiles:
                span = min(cfg.span_tiles, n_alltiles - n0)
                xk = xkp.tile([P, 2, cfg.span_tiles * P], F16, tag="xk")
                for kt in range(2):
                    nc.sync.dma_start(
                        out=xk[:, kt, 0 : span * P],
                        in_=xT_d[kt * P : (kt + 1) * P, n0 * P : (n0 + span) * P],
                    )
                g0 = 0
                while g0 < span:
                    grp = min(GRP, span - g0)
                    aug = auggp.tile([P, GRP, AUGW], F16, tag="aug")
                    for g in range(grp):
                        nt = g0 + g
                        gtile = n0 + nt  # global build tile index
                        ps = bldps.tile([P, USEW], F32, tag="bld")
                        for kt in range(2):
                            nc.tensor.matmul(
                                out=ps[:],
                                lhsT=xk[:, kt, nt * P : (nt + 1) * P],
                                rhs=wb[:, kt, :],
                                start=(kt == 0),
                                stop=(kt == 1),
                            )
                        nc.scalar.activation(
                            out=aug[:, g, 0:USEW],
                            in_=ps[:],
                            func=mybir.ActivationFunctionType.Copy,
                        )
                        if gtile < ntiles:
                            nc.vector.tensor_copy(
                                out=fb_all[:, gtile, :], in_=ps[:, 0 : 2 * OUT_DIM]
                            )
                            nc.vector.tensor_copy(
                                out=s_all[:, gtile, :], in_=ps[:, 258:260]
                            )
                    r0 = (n0 + g0) * P
                    nc.sync.dma_start(
                        out=whaug_d[r0 : r0 + grp * P, :].rearrange(
                            "(g p) c -> p g c", p=P
                        ),
                        in_=aug[:, 0:grp, :],
                    )
                    g0 += grp
                n0 += span

            _mark("p1_end")

            # ---------------- phase 2 ----------------
            OGRP = 8

            def compute_tile(t):
                """Emit gather + score + aggregation for tile t; returns
                state consumed by finalize_tile."""
                gall = gallp.tile([P, C, USEW], F16, tag="gall")
                for c in range(C):
                    nc.gpsimd.indirect_dma_start(
                        out=gall[:, c, :],
                        out_offset=None,
                        in_=whaug_d[:, :],
                        in_offset=IndirectOffsetOnAxis(
                            ap=dstI_t[:, t * C + c : t * C + c + 1], axis=0
                        ),
                    )
                # srcL rows broadcast to all partitions via stride-0 DMA,
                # then oneT [slot, (c p)] via iota compare
                srcr = srcrp.tile([P, C, P], F16, tag="srcr")
                sl = srcR_d[0:1, t * C * P : (t + 1) * C * P]
                nc.sync.dma_start(
                    out=srcr[:],
                    in_=bass.AP(sl.tensor, sl.offset, [[0, P], [1, C * P]]),
                )
                oneT = onehp.tile([P, C, P], F16, tag="oneT")
                nc.vector.tensor_tensor(
                    out=oneT[:],
                    in0=_ap_expand(iotac_t[:], [(0, C), (0, P)]),
                    in1=srcr[:],
                    op=mybir.AluOpType.is_equal,
                )
                s_ps = spsp.tile([P, C, 2], F32, tag="s_ps")
                for c in range(C):
                    nc.tensor.matmul(
                        out=s_ps[:, c, :],
                        lhsT=oneT[:, c, :],
                        rhs=s_all[:, t, :],
                        start=True,
                        stop=True,
                    )
                # e = lrelu(s + t); p = exp(e - SHIFT)
                e_t = finp.tile([P, C * 2], F32, tag="e_t")
                nc.vector.tensor_tensor(
                    out=e_t[:],
                    in0=s_ps[:].rearrange("p c h -> p (c h)"),
                    in1=gall[:, :, 256:258],
                    op=mybir.AluOpType.add,
                )
                e_s = finp.tile([P, C * 2], F32, tag="e_s")
                nc.vector.tensor_scalar(
                    out=e_s[:], in0=e_t[:], scalar1=0.2, scalar2=None,
                    op0=mybir.AluOpType.mult,
                )
                lr_t = finp.tile([P, C * 2], F32, tag="lr_t")
                nc.vector.tensor_tensor(
                    out=lr_t[:], in0=e_t[:], in1=e_s[:], op=mybir.AluOpType.max
                )
                p16 = finp.tile([P, C, 2], F16, tag="p16")
                nc.scalar.activation(
                    out=p16[:].rearrange("p c h -> p (c h)"),
                    in_=lr_t[:],
                    func=mybir.ActivationFunctionType.Exp,
                    bias=shift_t[:, 0:1],
                )
                # one-hot [e, slot] for aggregation
                oneh = onehp.tile([P, C, P], F16, tag="oneh")
                nc.vector.tensor_tensor(
                    out=oneh[:],
                    in0=_ap_expand(srcL_t[:, t * C : (t + 1) * C], [(1, C), (0, P)]),
                    in1=_ap_expand(iota16_t[:], [(0, C), (1, P)]),
                    op=mybir.AluOpType.is_equal,
                )
                # rhs = [Wh0*p0 | Wh1*p1 | p0 | p1]
                rhs = rhsp.tile([P, C, RHSW], F16, tag="rhs")
                nc.vector.tensor_tensor(
                    out=rhs[:, :, 0 : 2 * OUT_DIM],
                    in0=gall[:, :, 0 : 2 * OUT_DIM],
                    in1=_ap_expand(p16[:], [(2, C), (1, 2), (0, OUT_DIM)]),
                    op=mybir.AluOpType.mult,
                )
                nc.vector.tensor_copy(
                    out=rhs[:, :, 2 * OUT_DIM : RHSW], in_=p16[:]
                )
                ps = aggps.tile([P, RHSW], F32, tag="agg")
                for c in range(C):
                    nc.tensor.matmul(
                        out=ps[:],
                        lhsT=oneh[:, c, :],
                        rhs=rhs[:, c, :],
                        start=(c == 0),
                        stop=(c == C - 1),
                    )
                return ps

            def finalize_tile(t, ps, og, g):
                # den2 = den + DELTA  (Pool)
                den2 = finp.tile([P, 2], F32, tag="den2")
                nc.vector.tensor_scalar(
                    out=den2[:], in0=ps[:, 2 * OUT_DIM : RHSW], scalar1=DELTA,
                    scalar2=None, op0=mybir.AluOpType.add,
                )
                rcp = finp.tile([P, 2], F32, tag="rcp")
                nc.vector.reciprocal(out=rcp[:], in_=den2[:])
                # num2 = num + DELTA * fb   (DVE, fused)
                num2 = finp.tile([P, 2 * OUT_DIM], F32, tag="num2")
                nc.vector.scalar_tensor_tensor(
                    out=num2[:],
                    in0=fb_all[:, t, :],
                    scalar=DELTA,
                    in1=ps[:, 0 : 2 * OUT_DIM],
                    op0=mybir.AluOpType.mult,
                    op1=mybir.AluOpType.add,
                )
                for h in range(2):
                    nc.scalar.activation(
                        out=og[:, g, h * OUT_DIM : (h + 1) * OUT_DIM],
                        in_=num2[:, h * OUT_DIM : (h + 1) * OUT_DIM],
                        func=mybir.ActivationFunctionType.Copy,
                        scale=rcp[:, h : h + 1],
                    )

            tiles = [tt for _ in range(cfg.reps) for tt in range(ntiles)]
            pend = None  # (t, ps)
            og = None
            for t in tiles:
                ps = compute_tile(t)
                if pend is not None:
                    tf, psf = pend
                    g = tf % OGRP
                    if g == 0:
                        og = ogp.tile([P, OGRP, 2 * OUT_DIM], F32, tag="og")
                    finalize_tile(tf, psf, og, g)
                    if g == OGRP - 1 or tf == ntiles - 1:
                        t0 = tf - g
                        nc.sync.dma_start(
                            out=out_d[t0 * P : (tf + 1) * P, :].rearrange(
                                "(g p) c -> p g c", p=P
                            ),
                            in_=og[:, 0 : g + 1, :],
                        )
                pend = (t, ps)
            # last tile
            tf, psf = pend
            g = tf % OGRP
            if g == 0:
                og = ogp.tile([P, OGRP, 2 * OUT_DIM], F32, tag="og")
            finalize_tile(tf, psf, og, g)
            t0 = tf - g
            nc.sync.dma_start(
                out=out_d[t0 * P : (tf + 1) * P, :].rearrange("(g p) c -> p g c", p=P),
                in_=og[:, 0 : g + 1, :],
            )
            _mark("p2_end")

    nc.compile()
    return nc


_prog_cache = {}


def kernel(x, edge_index, W_w, W_b, a):
    cfg, shared, per_core = host_prep(x, edge_index, W_w, W_b, a, n_cores=8)
    if cfg not in _prog_cache:
        _prog_cache[cfg] = build_program(cfg)
    nc = _prog_cache[cfg]
    in_maps = [
        {kk: v for kk, v in {**shared, **pc}.items() if not kk.startswith("_")}
        for pc in per_core
    ]
    res = run_bass_kernel_spmd(nc, in_maps, list(range(cfg.n_cores)))
    out = np.zeros((cfg.n_nodes, 2 * OUT_DIM), dtype=np.float32)
    for k in range(cfg.n_cores):
        pc = per_core[k]
        out[pc["_nodes"]] = res.results[k]["out"][pc["_rows"]]
    return out


# revision 15
# speedup vs baseline: 1.2313x; 1.1156x over previous
"""GAT layer kernel for Trainium2, 8 NeuronCores.

Strategy (src-range sharding, no collectives):
  - Host: LPT-balance src nodes over 392 global (core, tile) slots so each
    tile owns ~2048 edges (C=16 chunks of 128).  Table rows live in
    per-core ROTATED tile-slot space: core k's own nodes occupy rows
    [0, ntiles*128) so phase-2 fallback/s data comes straight from SBUF.
  - Phase 1 (device): whaug table [npad, 264] fp16 rows
    [Wh0(128) | Wh1(128) | t0 t1 s0 s1 | pad2] via x @ [W | wt | ws]
    matmuls; PSUM->fp16 conversion on the Activation engine (one copy per
    tile); own-tile rows also kept in SBUF (fb_all) + s-pairs (s_all).
  - Phase 2 (device), per tile:
      Pool : C indirect row-gathers (dst rows, 128 offsets each)
      PE   : srcL broadcast (ones @ srcL_row), C tiny s-expand matmuls,
             C aggregation matmuls (one-hot lhsT)
      DVE  : one-hot builds (oneh [e,slot], oneT [slot,e]), rhs scaling by
             p, tiny e-ops; reciprocal
      ACT  : exp, output scaling
    Softmax uses a global SHIFT (softmax-invariant); deg-0 fallback uses
    out = (num + d*Wh_own) / (den + d) with d=1e-30 (exact for den=0).
"""

import math
import sys
from dataclasses import dataclass

import numpy as np

sys.path.insert(0, "/opt/trn_rl_repo")

import concourse.bass as bass
import concourse.mybir as mybir
import concourse.tile as tile
from concourse import bacc
from concourse.bass import IndirectOffsetOnAxis
from concourse.bass_utils import run_bass_kernel_spmd

N_NODES = 50000
IN_DIM = 256
OUT_DIM = 128
NUM_HEADS = 2

P = 128
AUGW = 260  # table row: Wh0(128), Wh1(128), t0, t1, s0, s1
USEW = 260
RHSW = 258  # aggregation matmul width: 256 features + 2 denominator cols
SHIFT = 4.0
DELTA = 1e-30

F32 = mybir.dt.float32
F16 = mybir.dt.float16
I32 = mybir.dt.int32


@dataclass(frozen=True)
class Cfg:
    n_nodes: int
    n_cores: int
    C: int
    span_tiles: int = 16
    reps: int = 1
    npass: int = 4

    @property
    def nodes_per_core(self):
        return self.n_nodes // self.n_cores

    @property
    def ntiles(self):
        return (self.nodes_per_core + P - 1) // P

    @property
    def npad(self):
        return self.n_cores * self.ntiles * P


def _ap_expand(ap, dims):
    """Return an AP keeping ap's partition dim and replacing the free dims
    with `dims` = list of (step, count) pairs (element units)."""
    return bass.AP(ap.tensor, ap.offset, [list(ap.ap[0])] + [[s, c] for s, c in dims])


def host_prep(x, edge_index, W_w, W_b, a, n_cores=8):
    """Index/layout preprocessing + parameter folding."""
    x = np.asarray(x, dtype=np.float32)
    edge_index = np.asarray(edge_index)
    W_w = np.asarray(W_w, dtype=np.float32)
    W_b = np.asarray(W_b, dtype=np.float32)
    a = np.asarray(a, dtype=np.float32)
    assert np.abs(W_b).max() == 0.0, "nonzero bias not supported"

    n_nodes, in_dim = x.shape
    D = OUT_DIM
    n_edges = edge_index.shape[1]

    # wbig columns: [W (256) | wt0 wt1 | ws0 ws1]  ->  ps = [Wh0 Wh1 t0 t1 s0 s1]
    a_src, a_dst = a[:D], a[D:]
    ws0 = W_w[:, 0:D] @ a_src
    ws1 = W_w[:, D : 2 * D] @ a_src
    wt0 = W_w[:, 0:D] @ a_dst
    wt1 = W_w[:, D : 2 * D] @ a_dst
    wbig = np.concatenate(
        [W_w, wt0[:, None], wt1[:, None], ws0[:, None], ws1[:, None]], axis=1
    ).astype(np.float16)  # [in_dim, 260]

    src = np.asarray(edge_index[0], dtype=np.int64)
    dst = np.asarray(edge_index[1], dtype=np.int64)
    order = np.argsort(src, kind="stable")
    src_s = src[order]
    dst_s = dst[order].astype(np.int64)

    npc = n_nodes // n_cores
    ntiles = (npc + P - 1) // P

    # LPT: assign nodes to (core,tile,slot), balancing per-tile edge counts.
    import heapq

    ntile_tot = n_cores * ntiles
    deg_all = np.bincount(src, minlength=n_nodes)
    order_n = np.argsort(-deg_all, kind="stable")
    heap = [(0, t) for t in range(ntile_tot)]
    heapq.heapify(heap)
    fill = np.zeros(ntile_tot, dtype=np.int64)
    node_tile = np.zeros(n_nodes, dtype=np.int64)
    node_slot = np.zeros(n_nodes, dtype=np.int64)
    for n in order_n:
        while True:
            w, t = heapq.heappop(heap)
            if fill[t] < P:
                break
        node_tile[n] = t
        node_slot[n] = fill[t]
        fill[t] += 1
        if fill[t] < P:
            heapq.heappush(heap, (w + int(deg_all[n]), t))

    grow = node_tile * P + node_slot  # global tile-slot row of each node

    gtile = node_tile[src_s]
    pos = node_slot[src_s]
    # sort by (tile, rotated dst-row of the owning core) so chunk c of a
    # tile covers a dst-row prefix bound in that core's table write order
    # -> gathers can start before the whole table is written
    rows_pc_ = ntiles * P
    npad_ = n_cores * rows_pc_
    rot_key = (grow[dst_s] - (gtile // ntiles) * rows_pc_) % npad_
    order2 = np.lexsort((rot_key, gtile))
    gtile = gtile[order2]
    dst_s = dst_s[order2]
    pos = pos[order2]

    counts = np.bincount(gtile, minlength=ntile_tot)
    C = int(math.ceil(counts.max() / P))
    cfg = Cfg(n_nodes=n_nodes, n_cores=n_cores, C=C)
    npad = cfg.npad
    slots_per_tile = C * P

    # xT in global tile-slot space (empty slots -> 0 rows)
    xTslot = np.zeros((in_dim, npad), dtype=np.float16)
    xTslot[:, grow] = x.T.astype(np.float16)

    starts = np.zeros(ntile_tot, dtype=np.int64)
    starts[1:] = np.cumsum(counts)[:-1]
    slot_in_tile = np.arange(n_edges) - starts[gtile]

    # per-edge-slot arrays in (tile, chunk, partition) space
    dstG = np.full((ntile_tot, slots_per_tile), -1, dtype=np.int64)  # dst grow
    srcL = np.full((ntile_tot, slots_per_tile), 255, dtype=np.uint8)
    flat = gtile * slots_per_tile + slot_in_tile
    dstG.reshape(-1)[flat] = grow[dst_s]
    srcL.reshape(-1)[flat] = pos.astype(np.uint8)

    def to_core_layout(arr, fillval):
        # [tiles, C*P] -> per-core [P, ntiles*C]
        a4 = arr.reshape(n_cores, ntiles, C, P)
        return np.ascontiguousarray(np.transpose(a4, (0, 3, 1, 2))).reshape(
            n_cores, P, ntiles * C
        )

    dstG_c = to_core_layout(dstG, 0)
    srcL_c = to_core_layout(srcL, -1.0)

    # Rtab[t, c] = 1 + max over cores of the rotated dst row in chunk c of
    # tile t (pad slots have dstG=0 -> rotated row npad-shift; make pads
    # point at row 0 instead so they don't inflate the bound)

    iota16 = np.broadcast_to(
        np.arange(P, dtype=np.uint8), (P, P)
    ).copy()  # iota16[p, j] = j
    iota_col = np.arange(P, dtype=np.uint8).reshape(P, 1).copy()

    shared = {"wbig": wbig, "iota16": iota16, "iotac": iota_col}
    shared["_Rtab"] = None  # placeholder, set below
    per_core = []
    rows_pc = ntiles * P
    # srcL row layout for the PE broadcast: [1, ntiles*C*P]
    Rtab = np.zeros((ntiles, C), dtype=np.int64)
    dstI_all = []
    for k in range(n_cores):
        shift = k * rows_pc
        rot = (dstG_c[k] - shift) % npad
        rot[dstG_c[k] < 0] = 0  # pad slots -> row 0
        dstI_all.append(rot.astype(np.int32))
        r3 = rot.reshape(P, ntiles, C)
        Rtab = np.maximum(Rtab, r3.max(axis=0))
    Rtab = Rtab + 1
    for k in range(n_cores):
        shift = k * rows_pc
        xT_k = np.roll(xTslot, -shift, axis=1)
        dstI_k = dstI_all[k]
        srcL_k = srcL_c[k]
        # row layout value at (t, c, p) = srcL_k[p, t*C + c]
        srcR_k = np.ascontiguousarray(
            np.transpose(srcL_k.reshape(P, ntiles, C), (1, 2, 0))
        ).reshape(1, ntiles * C * P)
        mine = (node_tile >= k * ntiles) & (node_tile < (k + 1) * ntiles)
        nodes_k = np.nonzero(mine)[0]
        rows_k = (node_tile[nodes_k] - k * ntiles) * P + node_slot[nodes_k]
        per_core.append(
            {
                "xT": xT_k,
                "dstI": dstI_k,
                "srcL": srcL_k,
                "srcR": srcR_k,
                "_nodes": nodes_k,
                "_rows": rows_k,
            }
        )
    shared["_Rtab"] = Rtab
    return cfg, shared, per_core


def build_program(cfg: Cfg, rtab, marks=None):
    """rtab: [ntiles, C] int array; rtab[t][c] = exclusive upper bound on
    table rows referenced by chunk c of tile t (edges are dst-sorted within
    each tile, so this is a prefix bound enabling gather/build overlap)."""
    C, ntiles, npad = cfg.C, cfg.ntiles, cfg.npad
    NP_ = cfg.npass
    CP = (C + NP_ - 1) // NP_

    nc = bacc.Bacc("TRN2", target_bir_lowering=False, debug=False)

    def _mark(label):
        if marks is not None:
            marks[label] = sum(len(b.instructions) for b in nc.m.functions[0].blocks)

    U8 = mybir.dt.uint8

    xT_d = nc.dram_tensor("xT", [IN_DIM, npad], F16, kind="ExternalInput")
    wbig_d = nc.dram_tensor("wbig", [IN_DIM, USEW], F16, kind="ExternalInput")
    iota16_d = nc.dram_tensor("iota16", [P, P], U8, kind="ExternalInput")
    iotac_d = nc.dram_tensor("iotac", [P, 1], U8, kind="ExternalInput")
    dstI_d = nc.dram_tensor("dstI", [P, ntiles * C], I32, kind="ExternalInput")
    srcL_d = nc.dram_tensor("srcL", [P, ntiles * C], U8, kind="ExternalInput")
    srcR_d = nc.dram_tensor("srcR", [1, ntiles * C * P], U8, kind="ExternalInput")
    out_d = nc.dram_tensor("out", [ntiles * P, 2 * OUT_DIM], F32, kind="ExternalOutput")

    whaug_d = nc.dram_tensor("whaug", [npad, AUGW], F16)
    whaug_ref = whaug_d[:, :]

    n_alltiles = npad // P

    with tile.TileContext(nc) as tc:
        with (
            tc.tile_pool(name="const", bufs=1) as constp,
            tc.tile_pool(name="xk", bufs=2) as xkp,
            tc.tile_pool(name="bld_ps", bufs=2, space="PSUM") as bldps,
            tc.tile_pool(name="augg", bufs=2) as auggp,
            tc.tile_pool(name="own", bufs=1) as ownp,
            tc.tile_pool(name="gall", bufs=2) as gallp,
            tc.tile_pool(name="srcr", bufs=2) as srcrp,
            tc.tile_pool(name="oneh", bufs=2) as onehp,
            tc.tile_pool(name="rhs", bufs=2) as rhsp,
            tc.tile_pool(name="s_ps", bufs=2, space="PSUM") as spsp,
            tc.tile_pool(name="agg_ps", bufs=2, space="PSUM") as aggps,
            tc.tile_pool(name="fin", bufs=2) as finp,
            tc.tile_pool(name="og", bufs=2) as ogp,
        ):
            # ---------------- constants ----------------
            wb = constp.tile([P, 2, USEW], F16, tag="wb")
            nc.sync.dma_start(
                out=wb[:], in_=wbig_d[:, :].rearrange("(kt kp) c -> kp kt c", kp=P)
            )
            iota16_t = constp.tile([P, P], U8, tag="iota16")
            nc.sync.dma_start(out=iota16_t[:], in_=iota16_d[:, :])
            iotac_t = constp.tile([P, 1], U8, tag="iotac")
            nc.sync.dma_start(out=iotac_t[:], in_=iotac_d[:, :])
            dstI_t = constp.tile([P, ntiles * C], I32, tag="dstI")
            nc.sync.dma_start(out=dstI_t[:], in_=dstI_d[:, :])
            srcL_t = constp.tile([P, ntiles * C], U8, tag="srcL")
            nc.sync.dma_start(out=srcL_t[:], in_=srcL_d[:, :])
            shift_t = constp.tile([P, 1], F32, tag="shift")
            nc.vector.memset(shift_t[:], -SHIFT)
            fb_all = ownp.tile([P, ntiles, 2 * OUT_DIM], F16, tag="fb_all")
            s_all = ownp.tile([P, ntiles, 2], F16, tag="s_all")
            s16_all = ownp.tile([P, ntiles, C, 2], F16, tag="s16_all")
            agg_sb = ownp.tile([P, ntiles, RHSW], F32, tag="agg_sb")

            _mark("consts_end")

            # ---------------- phase 1: build whaug table ----------------
            GRP = 4
            n0 = 0
            while n0 < n_alltiles:
                span = min(cfg.span_tiles, n_alltiles - n0)
                xk = xkp.tile([P, 2, cfg.span_tiles * P], F16, tag="xk")
                for kt in range(2):
                    nc.sync.dma_start(
                        out=xk[:, kt, 0 : span * P],
                        in_=xT_d[kt * P : (kt + 1) * P, n0 * P : (n0 + span) * P],
                    )
                g0 = 0
                while g0 < span:
                    grp = min(GRP, span - g0)
                    aug = auggp.tile([P, GRP, AUGW], F16, tag="aug")
                    for g in range(grp):
                        nt = g0 + g
                        gtile = n0 + nt
                        ps = bldps.tile([P, USEW], F32, tag="bld")
                        for kt in range(2):
                            nc.tensor.matmul(
                                out=ps[:],
                                lhsT=xk[:, kt, nt * P : (nt + 1) * P],
                                rhs=wb[:, kt, :],
                                start=(kt == 0),
                                stop=(kt == 1),
                            )
                        nc.scalar.activation(
                            out=aug[:, g, 0:USEW],
                            in_=ps[:],
                            func=mybir.ActivationFunctionType.Copy,
                        )
                        if gtile < ntiles:
                            nc.vector.tensor_copy(
                                out=fb_all[:, gtile, :], in_=ps[:, 0 : 2 * OUT_DIM]
                            )
                            nc.vector.tensor_copy(
                                out=s_all[:, gtile, :], in_=ps[:, 258:260]
                            )
                    r0 = (n0 + g0) * P
                    nc.sync.dma_start(
                        out=whaug_d[r0 : r0 + grp * P, :].rearrange(
                            "(g p) c -> p g c", p=P
                        ),
                        in_=aug[:, 0:grp, :],
                    )
                    g0 += grp
                n0 += span

            _mark("p1_end")

            # ------------- s-expansion for all tiles (overlaps phase 1) -------------
            for t in range(ntiles):
                srcr = srcrp.tile([P, C, P], U8, tag="srcr")
                sl = srcR_d[0:1, t * C * P : (t + 1) * C * P]
                nc.sync.dma_start(
                    out=srcr[:],
                    in_=bass.AP(sl.tensor, sl.offset, [[0, P], [1, C * P]]),
                )
                oneT = onehp.tile([P, C, P], F16, tag="oneT")
                nc.vector.tensor_tensor(
                    out=oneT[:],
                    in0=_ap_expand(iotac_t[:], [(0, C), (0, P)]),
                    in1=srcr[:],
                    op=mybir.AluOpType.is_equal,
                )
                s_ps = spsp.tile([P, C, 2], F32, tag="s_ps")
                for c in range(C):
                    nc.tensor.matmul(
                        out=s_ps[:, c, :],
                        lhsT=oneT[:, c, :],
                        rhs=s_all[:, t, :],
                        start=True,
                        stop=True,
                    )
                nc.vector.tensor_copy(out=s16_all[:, t, :, :], in_=s_ps[:])

            _mark("sexp_end")

            # ---------------- phase 2: passes of CP chunks ----------------
            OGRP = 8

            def compute(t, p):
                c0 = p * CP
                c1 = min(C, c0 + CP)
                nch = c1 - c0
                gall = gallp.tile([P, CP, USEW], F16, tag="gall")
                for c in range(c0, c1):
                    R = int(rtab[t][c])
                    bound = bass.AP(
                        whaug_ref.tensor, 0, [[AUGW, R], [1, USEW]]
                    )
                    nc.gpsimd.indirect_dma_start(
                        out=gall[:, c - c0, :],
                        out_offset=None,
                        in_=bound,
                        in_offset=IndirectOffsetOnAxis(
                            ap=dstI_t[:, t * C + c : t * C + c + 1], axis=0
                        ),
                    )
                # e = lrelu(s + t); pexp = exp(e - SHIFT)
                e_t = finp.tile([P, CP * 2], F32, tag="e_t")
                nc.vector.tensor_tensor(
                    out=e_t[:, 0 : nch * 2],
                    in0=s16_all[:, t, c0:c1, :],
                    in1=gall[:, 0:nch, 256:258],
                    op=mybir.AluOpType.add,
                )
                e_s = finp.tile([P, CP * 2], F32, tag="e_s")
                nc.vector.tensor_scalar(
                    out=e_s[:, 0 : nch * 2], in0=e_t[:, 0 : nch * 2],
                    scalar1=0.2, scalar2=None, op0=mybir.AluOpType.mult,
                )
                lr_t = finp.tile([P, CP * 2], F32, tag="lr_t")
                nc.vector.tensor_tensor(
                    out=lr_t[:, 0 : nch * 2], in0=e_t[:, 0 : nch * 2],
                    in1=e_s[:, 0 : nch * 2], op=mybir.AluOpType.max,
                )
                p16 = finp.tile([P, CP, 2], F16, tag="p16")
                nc.scalar.activation(
                    out=p16[:, 0:nch, :].rearrange("p c h -> p (c h)"),
                    in_=lr_t[:, 0 : nch * 2],
                    func=mybir.ActivationFunctionType.Exp,
                    bias=shift_t[:, 0:1],
                )
                oneh = onehp.tile([P, CP, P], F16, tag="oneh")
                nc.vector.tensor_tensor(
                    out=oneh[:, 0:nch, :],
                    in0=_ap_expand(srcL_t[:, t * C + c0 : t * C + c1], [(1, nch), (0, P)]),
                    in1=_ap_expand(iota16_t[:], [(0, nch), (1, P)]),
                    op=mybir.AluOpType.is_equal,
                )
                rhs = rhsp.tile([P, CP, RHSW], F16, tag="rhs")
                nc.vector.tensor_tensor(
                    out=rhs[:, 0:nch, 0 : 2 * OUT_DIM],
                    in0=gall[:, 0:nch, 0 : 2 * OUT_DIM],
                    in1=_ap_expand(p16[:], [(2, nch), (1, 2), (0, OUT_DIM)]),
                    op=mybir.AluOpType.mult,
                )
                nc.vector.tensor_copy(
                    out=rhs[:, 0:nch, 2 * OUT_DIM : RHSW], in_=p16[:, 0:nch, :]
                )
                ps = aggps.tile([P, RHSW], F32, tag="agg")
                for c in range(nch):
                    nc.tensor.matmul(
                        out=ps[:],
                        lhsT=oneh[:, c, :],
                        rhs=rhs[:, c, :],
                        start=(c == 0),
                        stop=(c == nch - 1),
                    )
                if p == 0:
                    nc.vector.tensor_copy(out=agg_sb[:, t, :], in_=ps[:])
                else:
                    nc.vector.tensor_tensor(
                        out=agg_sb[:, t, :], in0=agg_sb[:, t, :], in1=ps[:],
                        op=mybir.AluOpType.add,
                    )

            def finalize(t, og, g):
                den2 = finp.tile([P, 2], F32, tag="den2")
                nc.vector.tensor_scalar(
                    out=den2[:], in0=agg_sb[:, t, 2 * OUT_DIM : RHSW],
                    scalar1=DELTA, scalar2=None, op0=mybir.AluOpType.add,
                )
                rcp = finp.tile([P, 2], F32, tag="rcp")
                nc.vector.reciprocal(out=rcp[:], in_=den2[:])
                num2 = finp.tile([P, 2 * OUT_DIM], F32, tag="num2")
                nc.vector.scalar_tensor_tensor(
                    out=num2[:],
                    in0=fb_all[:, t, :],
                    scalar=DELTA,
                    in1=agg_sb[:, t, 0 : 2 * OUT_DIM],
                    op0=mybir.AluOpType.mult,
                    op1=mybir.AluOpType.add,
                )
                for h in range(2):
                    nc.scalar.activation(
                        out=og[:, g, h * OUT_DIM : (h + 1) * OUT_DIM],
                        in_=num2[:, h * OUT_DIM : (h + 1) * OUT_DIM],
                        func=mybir.ActivationFunctionType.Copy,
                        scale=rcp[:, h : h + 1],
                    )

            for rep in range(cfg.reps):
                for p in range(NP_):
                    for t in range(ntiles):
                        compute(t, p)
                # finalize with one-tile software pipelining on the last pass
                og = None
                pend = None
                for t in range(ntiles):
                    if pend is not None:
                        tf = pend
                        g = tf % OGRP
                        if g == 0:
                            og = ogp.tile([P, OGRP, 2 * OUT_DIM], F32, tag="og")
                        finalize(tf, og, g)
                        if g == OGRP - 1 or tf == ntiles - 1:
                            t0 = tf - g
                            nc.sync.dma_start(
                                out=out_d[t0 * P : (tf + 1) * P, :].rearrange(
                                    "(g p) c -> p g c", p=P
                                ),
                                in_=og[:, 0 : g + 1, :],
                            )
                    pend = t
                tf = pend
                g = tf % OGRP
                if g == 0:
                    og = ogp.tile([P, OGRP, 2 * OUT_DIM], F32, tag="og")
                finalize(tf, og, g)
                t0 = tf - g
                nc.sync.dma_start(
                    out=out_d[t0 * P : (tf + 1) * P, :].rearrange(
                        "(g p) c -> p g c", p=P
                    ),
                    in_=og[:, 0 : g + 1, :],
                )
            _mark("p2_end")

    nc.compile()
    return nc


_prog_cache = {}


def kernel(x, edge_index, W_w, W_b, a):
    cfg, shared, per_core = host_prep(x, edge_index, W_w, W_b, a, n_cores=8)
    rtab = shared["_Rtab"]
    key = (cfg, rtab.tobytes())
    if key not in _prog_cache:
        _prog_cache[key] = build_program(cfg, rtab)
    nc = _prog_cache[key]
    in_maps = [
        {kk: v for kk, v in {**shared, **pc}.items() if not kk.startswith("_")}
        for pc in per_core
    ]
    res = run_bass_kernel_spmd(nc, in_maps, list(range(cfg.n_cores)))
    out = np.zeros((cfg.n_nodes, 2 * OUT_DIM), dtype=np.float32)
    for k in range(cfg.n_cores):
        pc = per_core[k]
        out[pc["_nodes"]] = res.results[k]["out"][pc["_rows"]]
    return out


# revision 22
# speedup vs baseline: 1.6546x; 1.3439x over previous
"""GAT layer kernel for Trainium2, 8 NeuronCores.

Strategy (src-range sharding, no collectives):
  - Host: LPT-balance src nodes over 392 global (core, tile) slots so each
    tile owns ~2048 edges (C=16 chunks of 128).  Table rows live in
    per-core ROTATED tile-slot space: core k's own nodes occupy rows
    [0, ntiles*128) so phase-2 fallback/s data comes straight from SBUF.
  - Phase 1 (device): whaug table [npad, 264] fp16 rows
    [Wh0(128) | Wh1(128) | t0 t1 s0 s1 | pad2] via x @ [W | wt | ws]
    matmuls; PSUM->fp16 conversion on the Activation engine (one copy per
    tile); own-tile rows also kept in SBUF (fb_all) + s-pairs (s_all).
  - Phase 2 (device), per tile:
      Pool : C indirect row-gathers (dst rows, 128 offsets each)
      PE   : srcL broadcast (ones @ srcL_row), C tiny s-expand matmuls,
             C aggregation matmuls (one-hot lhsT)
      DVE  : one-hot builds (oneh [e,slot], oneT [slot,e]), rhs scaling by
             p, tiny e-ops; reciprocal
      ACT  : exp, output scaling
    Softmax uses a global SHIFT (softmax-invariant); deg-0 fallback uses
    out = (num + d*Wh_own) / (den + d) with d=1e-30 (exact for den=0).
"""

import math
import sys
from dataclasses import dataclass

import numpy as np

sys.path.insert(0, "/opt/trn_rl_repo")

import concourse.bass as bass
import concourse.mybir as mybir
import concourse.tile as tile
from concourse import bacc
from concourse.bass import IndirectOffsetOnAxis
from concourse.bass_utils import run_bass_kernel_spmd

N_NODES = 50000
IN_DIM = 256
OUT_DIM = 128
NUM_HEADS = 2

P = 128
AUGW = 260  # table row: Wh0(128), Wh1(128), t0, t1, s0, s1
USEW = 260
RHSW = 258  # aggregation matmul width: 256 features + 2 denominator cols
SHIFT = 4.0
DELTA = 1e-30

F32 = mybir.dt.float32
F16 = mybir.dt.float16
I32 = mybir.dt.int32


@dataclass(frozen=True)
class Cfg:
    n_nodes: int
    n_cores: int
    C: int
    span_tiles: int = 16
    reps: int = 1
    npass: int = 4

    @property
    def nodes_per_core(self):
        return self.n_nodes // self.n_cores

    @property
    def ntiles(self):
        return (self.nodes_per_core + P - 1) // P

    @property
    def npad(self):
        return self.n_cores * self.ntiles * P


def _ap_expand(ap, dims):
    """Return an AP keeping ap's partition dim and replacing the free dims
    with `dims` = list of (step, count) pairs (element units)."""
    return bass.AP(ap.tensor, ap.offset, [list(ap.ap[0])] + [[s, c] for s, c in dims])


def host_prep(x, edge_index, W_w, W_b, a, n_cores=8):
    """Index/layout preprocessing + parameter folding."""
    x = np.asarray(x, dtype=np.float32)
    edge_index = np.asarray(edge_index)
    W_w = np.asarray(W_w, dtype=np.float32)
    W_b = np.asarray(W_b, dtype=np.float32)
    a = np.asarray(a, dtype=np.float32)
    assert np.abs(W_b).max() == 0.0, "nonzero bias not supported"

    n_nodes, in_dim = x.shape
    D = OUT_DIM
    n_edges = edge_index.shape[1]

    # wbig columns: [W (256) | wt0 wt1 | ws0 ws1]  ->  ps = [Wh0 Wh1 t0 t1 s0 s1]
    a_src, a_dst = a[:D], a[D:]
    ws0 = W_w[:, 0:D] @ a_src
    ws1 = W_w[:, D : 2 * D] @ a_src
    wt0 = W_w[:, 0:D] @ a_dst
    wt1 = W_w[:, D : 2 * D] @ a_dst
    wbig = np.concatenate(
        [W_w, wt0[:, None], wt1[:, None], ws0[:, None], ws1[:, None]], axis=1
    ).astype(np.float16)  # [in_dim, 260]

    src = np.asarray(edge_index[0], dtype=np.int64)
    dst = np.asarray(edge_index[1], dtype=np.int64)
    order = np.argsort(src, kind="stable")
    src_s = src[order]
    dst_s = dst[order].astype(np.int64)

    npc = n_nodes // n_cores
    ntiles = (npc + P - 1) // P

    # LPT: assign nodes to (core,tile,slot), balancing per-tile edge counts.
    import heapq

    ntile_tot = n_cores * ntiles
    deg_all = np.bincount(src, minlength=n_nodes)
    order_n = np.argsort(-deg_all, kind="stable")
    heap = [(0, t) for t in range(ntile_tot)]
    heapq.heapify(heap)
    fill = np.zeros(ntile_tot, dtype=np.int64)
    node_tile = np.zeros(n_nodes, dtype=np.int64)
    node_slot = np.zeros(n_nodes, dtype=np.int64)
    for n in order_n:
        while True:
            w, t = heapq.heappop(heap)
            if fill[t] < P:
                break
        node_tile[n] = t
        node_slot[n] = fill[t]
        fill[t] += 1
        if fill[t] < P:
            heapq.heappush(heap, (w + int(deg_all[n]), t))

    grow = node_tile * P + node_slot  # global tile-slot row of each node

    gtile = node_tile[src_s]
    pos = node_slot[src_s]
    # sort by (tile, rotated dst-row of the owning core) so chunk c of a
    # tile covers a dst-row prefix bound in that core's table write order
    # -> gathers can start before the whole table is written
    rows_pc_ = ntiles * P
    npad_ = n_cores * rows_pc_
    rot_key = (grow[dst_s] - (gtile // ntiles) * rows_pc_) % npad_
    order2 = np.lexsort((rot_key, gtile))
    gtile = gtile[order2]
    dst_s = dst_s[order2]
    pos = pos[order2]

    counts = np.bincount(gtile, minlength=ntile_tot)
    C = int(math.ceil(counts.max() / P))
    cfg = Cfg(n_nodes=n_nodes, n_cores=n_cores, C=C)
    npad = cfg.npad
    slots_per_tile = C * P

    # xT in global tile-slot space (empty slots -> 0 rows)
    xTslot = np.zeros((in_dim, npad), dtype=np.float16)
    xTslot[:, grow] = x.T.astype(np.float16)

    starts = np.zeros(ntile_tot, dtype=np.int64)
    starts[1:] = np.cumsum(counts)[:-1]
    slot_in_tile = np.arange(n_edges) - starts[gtile]

    # per-edge-slot arrays in (tile, chunk, partition) space
    dstG = np.full((ntile_tot, slots_per_tile), -1, dtype=np.int64)  # dst grow
    srcL = np.full((ntile_tot, slots_per_tile), 255, dtype=np.uint8)
    flat = gtile * slots_per_tile + slot_in_tile
    dstG.reshape(-1)[flat] = grow[dst_s]
    srcL.reshape(-1)[flat] = pos.astype(np.uint8)

    def to_core_layout(arr, fillval):
        # [tiles, C*P] -> per-core [P, ntiles*C]
        a4 = arr.reshape(n_cores, ntiles, C, P)
        return np.ascontiguousarray(np.transpose(a4, (0, 3, 1, 2))).reshape(
            n_cores, P, ntiles * C
        )

    dstG_c = to_core_layout(dstG, 0)
    srcL_c = to_core_layout(srcL, -1.0)

    # Rtab[t, c] = 1 + max over cores of the rotated dst row in chunk c of
    # tile t (pad slots have dstG=0 -> rotated row npad-shift; make pads
    # point at row 0 instead so they don't inflate the bound)

    iota16 = np.broadcast_to(
        np.arange(P, dtype=np.uint8), (P, P)
    ).copy()  # iota16[p, j] = j
    iota_col = np.arange(P, dtype=np.uint8).reshape(P, 1).copy()

    shared = {"wbig": wbig, "iota16": iota16, "iotac": iota_col}
    shared["_Rtab"] = None  # placeholder, set below
    per_core = []
    rows_pc = ntiles * P
    # srcL row layout for the PE broadcast: [1, ntiles*C*P]
    Rtab = np.zeros((ntiles, C), dtype=np.int64)
    dstI_all = []
    for k in range(n_cores):
        shift = k * rows_pc
        rot = (dstG_c[k] - shift) % npad
        rot[dstG_c[k] < 0] = 0  # pad slots -> row 0
        dstI_all.append(rot.astype(np.int32))
        r3 = rot.reshape(P, ntiles, C)
        Rtab = np.maximum(Rtab, r3.max(axis=0))
    Rtab = Rtab + 1
    for k in range(n_cores):
        shift = k * rows_pc
        xT_k = np.roll(xTslot, -shift, axis=1)
        dstI_k = dstI_all[k]
        srcL_k = srcL_c[k]
        # row layout value at (t, c, p) = srcL_k[p, t*C + c]
        srcR_k = np.ascontiguousarray(
            np.transpose(srcL_k.reshape(P, ntiles, C), (1, 2, 0))
        ).reshape(1, ntiles * C * P)
        mine = (node_tile >= k * ntiles) & (node_tile < (k + 1) * ntiles)
        nodes_k = np.nonzero(mine)[0]
        rows_k = (node_tile[nodes_k] - k * ntiles) * P + node_slot[nodes_k]
        per_core.append(
            {
                "xT": xT_k,
                "dstI": dstI_k,
                "srcL": srcL_k,
                "srcR": srcR_k,
                "_nodes": nodes_k,
                "_rows": rows_k,
            }
        )
    shared["_Rtab"] = Rtab
    return cfg, shared, per_core


def build_program(cfg: Cfg, rtab, marks=None):
    """rtab: [ntiles, C] int array; rtab[t][c] = exclusive upper bound on
    table rows referenced by chunk c of tile t (edges are dst-sorted within
    each tile, so this is a prefix bound enabling gather/build overlap)."""
    C, ntiles, npad = cfg.C, cfg.ntiles, cfg.npad
    NP_ = cfg.npass
    CP = (C + NP_ - 1) // NP_

    nc = bacc.Bacc("TRN2", target_bir_lowering=False, debug=False)

    def _mark(label):
        if marks is not None:
            marks[label] = sum(len(b.instructions) for b in nc.m.functions[0].blocks)

    U8 = mybir.dt.uint8

    xT_d = nc.dram_tensor("xT", [IN_DIM, npad], F16, kind="ExternalInput")
    wbig_d = nc.dram_tensor("wbig", [IN_DIM, USEW], F16, kind="ExternalInput")
    iota16_d = nc.dram_tensor("iota16", [P, P], U8, kind="ExternalInput")
    iotac_d = nc.dram_tensor("iotac", [P, 1], U8, kind="ExternalInput")
    dstI_d = nc.dram_tensor("dstI", [P, ntiles * C], I32, kind="ExternalInput")
    srcL_d = nc.dram_tensor("srcL", [P, ntiles * C], U8, kind="ExternalInput")
    srcR_d = nc.dram_tensor("srcR", [1, ntiles * C * P], U8, kind="ExternalInput")
    out_d = nc.dram_tensor("out", [ntiles * P, 2 * OUT_DIM], F32, kind="ExternalOutput")

    whaug_d = nc.dram_tensor("whaug", [npad, AUGW], F16)
    whaug_ref = whaug_d[:, :]

    n_alltiles = npad // P

    with tile.TileContext(nc) as tc:
        with (
            tc.tile_pool(name="const", bufs=1) as constp,
            tc.tile_pool(name="xk", bufs=2) as xkp,
            tc.tile_pool(name="bld_ps", bufs=2, space="PSUM") as bldps,
            tc.tile_pool(name="augg", bufs=2) as auggp,
            tc.tile_pool(name="own", bufs=1) as ownp,
            tc.tile_pool(name="gall", bufs=6) as gallp,
            tc.tile_pool(name="srcr", bufs=2) as srcrp,
            tc.tile_pool(name="oneh", bufs=4) as onehp,
            tc.tile_pool(name="rhs", bufs=4) as rhsp,
            tc.tile_pool(name="s_ps", bufs=2, space="PSUM") as spsp,
            tc.tile_pool(name="agg_ps", bufs=2, space="PSUM") as aggps,
            tc.tile_pool(name="fin", bufs=4) as finp,
            tc.tile_pool(name="og", bufs=2) as ogp,
        ):
            # ---------------- constants ----------------
            wb = constp.tile([P, 2, USEW], F16, tag="wb")
            nc.sync.dma_start(
                out=wb[:], in_=wbig_d[:, :].rearrange("(kt kp) c -> kp kt c", kp=P)
            )
            iota16_t = constp.tile([P, P], U8, tag="iota16")
            nc.sync.dma_start(out=iota16_t[:], in_=iota16_d[:, :])
            iotac_t = constp.tile([P, 1], U8, tag="iotac")
            nc.sync.dma_start(out=iotac_t[:], in_=iotac_d[:, :])
            dstI_t = constp.tile([P, ntiles * C], I32, tag="dstI")
            nc.sync.dma_start(out=dstI_t[:], in_=dstI_d[:, :])
            srcL_t = constp.tile([P, ntiles * C], U8, tag="srcL")
            nc.sync.dma_start(out=srcL_t[:], in_=srcL_d[:, :])
            shift_t = constp.tile([P, 1], F32, tag="shift")
            nc.vector.memset(shift_t[:], -SHIFT)
            fb_all = ownp.tile([P, ntiles, 2 * OUT_DIM], F16, tag="fb_all")
            s_all = ownp.tile([P, ntiles, 2], F16, tag="s_all")
            s16_all = ownp.tile([P, ntiles, C, 2], F16, tag="s16_all")
            agg_sb = ownp.tile([P, ntiles, RHSW], F32, tag="agg_sb")

            _mark("consts_end")

            # ---------------- phase 1: build whaug table ----------------
            def s_expand(t):
                srcr = srcrp.tile([P, C, P], U8, tag="srcr")
                sl = srcR_d[0:1, t * C * P : (t + 1) * C * P]
                nc.sync.dma_start(
                    out=srcr[:],
                    in_=bass.AP(sl.tensor, sl.offset, [[0, P], [1, C * P]]),
                )
                oneT = onehp.tile([P, C, P], F16, tag="oneT")
                nc.vector.tensor_tensor(
                    out=oneT[:],
                    in0=_ap_expand(iotac_t[:], [(0, C), (0, P)]),
                    in1=srcr[:],
                    op=mybir.AluOpType.is_equal,
                )
                s_ps = spsp.tile([P, C, 2], F32, tag="s_ps")
                for c in range(C):
                    nc.tensor.matmul(
                        out=s_ps[:, c, :],
                        lhsT=oneT[:, c, :],
                        rhs=s_all[:, t, :],
                        start=True,
                        stop=True,
                    )
                nc.vector.tensor_copy(out=s16_all[:, t, :, :], in_=s_ps[:])

            GRP = 4
            n0 = 0
            while n0 < n_alltiles:
                span = min(cfg.span_tiles, n_alltiles - n0)
                xk = xkp.tile([P, 2, cfg.span_tiles * P], F16, tag="xk")
                for kt in range(2):
                    nc.sync.dma_start(
                        out=xk[:, kt, 0 : span * P],
                        in_=xT_d[kt * P : (kt + 1) * P, n0 * P : (n0 + span) * P],
                    )
                g0 = 0
                while g0 < span:
                    grp = min(GRP, span - g0)
                    aug = auggp.tile([P, GRP, AUGW], F16, tag="aug")
                    for g in range(grp):
                        nt = g0 + g
                        gtile = n0 + nt
                        ps = bldps.tile([P, USEW], F32, tag="bld")
                        for kt in range(2):
                            nc.tensor.matmul(
                                out=ps[:],
                                lhsT=xk[:, kt, nt * P : (nt + 1) * P],
                                rhs=wb[:, kt, :],
                                start=(kt == 0),
                                stop=(kt == 1),
                            )
                        nc.scalar.activation(
                            out=aug[:, g, 0:USEW],
                            in_=ps[:],
                            func=mybir.ActivationFunctionType.Copy,
                        )
                        if gtile < ntiles:
                            nc.vector.tensor_copy(
                                out=fb_all[:, gtile, :], in_=ps[:, 0 : 2 * OUT_DIM]
                            )
                            nc.vector.tensor_copy(
                                out=s_all[:, gtile, :], in_=ps[:, 258:260]
                            )
                    r0 = (n0 + g0) * P
                    nc.sync.dma_start(
                        out=whaug_d[r0 : r0 + grp * P, :].rearrange(
                            "(g p) c -> p g c", p=P
                        ),
                        in_=aug[:, 0:grp, :],
                    )
                    g0 += grp
                n0 += span

            _mark("p1_end")

            for t in range(ntiles):
                s_expand(t)

            # ---------------- phase 2: passes of CP chunks ----------------
            OGRP = 8

            def compute(t, p):
                c0 = p * CP
                c1 = min(C, c0 + CP)
                nch = c1 - c0
                gall = gallp.tile([P, CP, USEW], F16, tag="gall")
                for c in range(c0, c1):
                    R = int(rtab[t][c])
                    bound = bass.AP(
                        whaug_ref.tensor, 0, [[AUGW, R], [1, USEW]]
                    )
                    nc.gpsimd.indirect_dma_start(
                        out=gall[:, c - c0, :],
                        out_offset=None,
                        in_=bound,
                        in_offset=IndirectOffsetOnAxis(
                            ap=dstI_t[:, t * C + c : t * C + c + 1], axis=0
                        ),
                    )
                # e = lrelu(s + t); pexp = exp(e - SHIFT)
                e_t = finp.tile([P, CP * 2], F32, tag="e_t")
                nc.vector.tensor_tensor(
                    out=e_t[:, 0 : nch * 2],
                    in0=s16_all[:, t, c0:c1, :],
                    in1=gall[:, 0:nch, 256:258],
                    op=mybir.AluOpType.add,
                )
                e_s = finp.tile([P, CP * 2], F32, tag="e_s")
                nc.vector.tensor_scalar(
                    out=e_s[:, 0 : nch * 2], in0=e_t[:, 0 : nch * 2],
                    scalar1=0.2, scalar2=None, op0=mybir.AluOpType.mult,
                )
                lr_t = finp.tile([P, CP * 2], F32, tag="lr_t")
                nc.vector.tensor_tensor(
                    out=lr_t[:, 0 : nch * 2], in0=e_t[:, 0 : nch * 2],
                    in1=e_s[:, 0 : nch * 2], op=mybir.AluOpType.max,
                )
                p16 = finp.tile([P, CP, 2], F16, tag="p16")
                nc.scalar.activation(
                    out=p16[:, 0:nch, :].rearrange("p c h -> p (c h)"),
                    in_=lr_t[:, 0 : nch * 2],
                    func=mybir.ActivationFunctionType.Exp,
                    bias=shift_t[:, 0:1],
                )
                oneh = onehp.tile([P, CP, P], F16, tag="oneh")
                nc.vector.tensor_tensor(
                    out=oneh[:, 0:nch, :],
                    in0=_ap_expand(srcL_t[:, t * C + c0 : t * C + c1], [(1, nch), (0, P)]),
                    in1=_ap_expand(iota16_t[:], [(0, nch), (1, P)]),
                    op=mybir.AluOpType.is_equal,
                )
                rhs = rhsp.tile([P, CP, RHSW], F16, tag="rhs")
                nc.vector.tensor_tensor(
                    out=rhs[:, 0:nch, 0 : 2 * OUT_DIM],
                    in0=gall[:, 0:nch, 0 : 2 * OUT_DIM],
                    in1=_ap_expand(p16[:], [(2, nch), (1, 2), (0, OUT_DIM)]),
                    op=mybir.AluOpType.mult,
                )
                nc.vector.tensor_copy(
                    out=rhs[:, 0:nch, 2 * OUT_DIM : RHSW], in_=p16[:, 0:nch, :]
                )
                ps = aggps.tile([P, RHSW], F32, tag="agg")
                for c in range(nch):
                    nc.tensor.matmul(
                        out=ps[:],
                        lhsT=oneh[:, c, :],
                        rhs=rhs[:, c, :],
                        start=(c == 0),
                        stop=(c == nch - 1),
                    )
                if p == 0:
                    nc.vector.tensor_copy(out=agg_sb[:, t, :], in_=ps[:])
                else:
                    nc.vector.tensor_tensor(
                        out=agg_sb[:, t, :], in0=agg_sb[:, t, :], in1=ps[:],
                        op=mybir.AluOpType.add,
                    )

            def finalize(t, og, g):
                den2 = finp.tile([P, 2], F32, tag="den2")
                nc.vector.tensor_scalar(
                    out=den2[:], in0=agg_sb[:, t, 2 * OUT_DIM : RHSW],
                    scalar1=DELTA, scalar2=None, op0=mybir.AluOpType.add,
                )
                rcp = finp.tile([P, 2], F32, tag="rcp")
                nc.vector.reciprocal(out=rcp[:], in_=den2[:])
                num2 = finp.tile([P, 2 * OUT_DIM], F32, tag="num2")
                nc.vector.scalar_tensor_tensor(
                    out=num2[:],
                    in0=fb_all[:, t, :],
                    scalar=DELTA,
                    in1=agg_sb[:, t, 0 : 2 * OUT_DIM],
                    op0=mybir.AluOpType.mult,
                    op1=mybir.AluOpType.add,
                )
                for h in range(2):
                    nc.scalar.activation(
                        out=og[:, g, h * OUT_DIM : (h + 1) * OUT_DIM],
                        in_=num2[:, h * OUT_DIM : (h + 1) * OUT_DIM],
                        func=mybir.ActivationFunctionType.Copy,
                        scale=rcp[:, h : h + 1],
                    )

            for rep in range(cfg.reps):
                for p in range(NP_ - 1):
                    for t in range(ntiles):
                        compute(t, p)
                # last pass: finalize tile t-1 after computing tile t
                og = None
                pend = None

                def flush(tf, og):
                    g = tf % OGRP
                    if g == OGRP - 1 or tf == ntiles - 1:
                        t0 = tf - g
                        nc.sync.dma_start(
                            out=out_d[t0 * P : (tf + 1) * P, :].rearrange(
                                "(g p) c -> p g c", p=P
                            ),
                            in_=og[:, 0 : g + 1, :],
                        )

                for t in range(ntiles):
                    compute(t, NP_ - 1)
                    if pend is not None:
                        g = pend % OGRP
                        if g == 0:
                            og = ogp.tile([P, OGRP, 2 * OUT_DIM], F32, tag="og")
                        finalize(pend, og, g)
                        flush(pend, og)
                    pend = t
                g = pend % OGRP
                if g == 0:
                    og = ogp.tile([P, OGRP, 2 * OUT_DIM], F32, tag="og")
                finalize(pend, og, g)
                flush(pend, og)
            _mark("p2_end")

    nc.compile()
    return nc


_prog_cache = {}


def kernel(x, edge_index, W_w, W_b, a):
    cfg, shared, per_core = host_prep(x, edge_index, W_w, W_b, a, n_cores=8)
    rtab = shared["_Rtab"]
    key = (cfg, rtab.tobytes())
    if key not in _prog_cache:
        _prog_cache[key] = build_program(cfg, rtab)
    nc = _prog_cache[key]
    in_maps = [
        {kk: v for kk, v in {**shared, **pc}.items() if not kk.startswith("_")}
        for pc in per_core
    ]
    res = run_bass_kernel_spmd(nc, in_maps, list(range(cfg.n_cores)))
    out = np.zeros((cfg.n_nodes, 2 * OUT_DIM), dtype=np.float32)
    for k in range(cfg.n_cores):
        pc = per_core[k]
        out[pc["_nodes"]] = res.results[k]["out"][pc["_rows"]]
    return out
